# revision 9
# baseline (speedup 1.0000x reference)
"""Trainium2 Bass kernel: patch-conv (Conv2d C3->E768, k4 s4) + giant linear.

y[b, eo] = sum_K flat[b, K] * wlin[eo, K] + blin[eo],
flat[b, e*256+p] = conv[b, e, p] (+ bconv[e]), K = 196608.

Reassociated algorithm (matrix-chain reordering, all FLOPs on device):
    y[b,eo] = sum_{p,cij} xp[cij,p,b] * W2[p,cij,eo]
    W2[p,cij,eo] = sum_e wca[cij,e] * wlin[eo, e*256+p]
where xp is the im2col of x (pure index remap, row 48 = ones) and wca is
wconv reshaped [49, E] (row 48 = bconv). This computes the identical
function with 19.7 GFLOP instead of 82 GFLOP, and reads wlin exactly once.

Sharding (8 cores): shard the patch index p (32 patches/core). Each core:
  - reads its wlin slice re-laid-out on host as wlp[e_r, (p, ech, eo)] bf16
    (37.75 MB -- the DMA roofline term),
  - W2-mm: for each p: 6 e-chunks x (N=512 + N=256) matmuls, stationary
    wce[e_chunk] = wcaT slice [128,49], accumulate PSUM [49,768],
  - copies W2_p to SBUF bf16 (vector/scalar alternating),
  - final-mm: stationary xpp[:, p, b-half] [49,128], moving W2_p, PSUM
    accumulates y-partial [128b, 512|256 eo] over the 32 p's.
  - Host: sum the 8 partial y's, add blin.
All PSUM tiles are bank-sized (2048B) so accumulation groups never cross
a PSUM bank: 4 banks y-accum + 2x2 banks W2-accum = 8.
"""

import numpy as np
import ml_dtypes

B, C, H, W = 256, 3, 64, 64
P, Hp, Wp, NP = 4, 16, 16, 256
E = 768
CIJ = 49                  # 48 conv taps + 1 bias row
NCORES = 8
PL = NP // NCORES         # 32 patches per core
NECH = E // 128           # 6 e-chunks in the W2 contraction
PCOL = NECH * E           # 4608 wlp columns per patch

_CACHE = {}


def _build_bass():
    import concourse.bass as bass
    import concourse.mybir as mybir
    import concourse.tile as tile
    from contextlib import ExitStack

    dt = mybir.dt
    nc = bass.Bass()
    wce_d = nc.dram_tensor("wce", [128, NECH * CIJ], dt.bfloat16, kind="ExternalInput")
    xpp_d = nc.dram_tensor("xpp", [CIJ, PL * B], dt.bfloat16, kind="ExternalInput")
    wlp_d = nc.dram_tensor("wlp", [128, PL * PCOL], dt.bfloat16, kind="ExternalInput")
    out_d = nc.dram_tensor("y", [B, E], dt.float32, kind="ExternalOutput")

    with tile.TileContext(nc) as tc, ExitStack() as ctx:
        singles = ctx.enter_context(tc.tile_pool(name="singles", bufs=1))
        # wce + xpp ride the sync queue so the gpsimd queue's first
        # instruction is already a wlp stream DMA (earliest possible start).
        wce = singles.tile([128, NECH * CIJ], dt.bfloat16)
        nc.sync.dma_start(out=wce[:], in_=wce_d[:])
        xpp = singles.tile([CIJ, PL * B], dt.bfloat16)
        nc.sync.dma_start(out=xpp[:, 0 : PL * B // 2], in_=xpp_d[:, 0 : PL * B // 2])
        nc.sync.dma_start(out=xpp[:, PL * B // 2 :], in_=xpp_d[:, PL * B // 2 :])

        # W2 staging ring: one [49, 768] bf16 tile per patch, recycled.
        w2_pool = ctx.enter_context(tc.tile_pool(name="w2ring", bufs=6))
        wlp_pool = ctx.enter_context(tc.tile_pool(name="wlp", bufs=5))
        out_pool = ctx.enter_context(tc.tile_pool(name="out_sb", bufs=1))

        # Warmup: absorb the wce DMA-ready wait on a throwaway matmul so the
        # first real W2 matmul carries only the wlp(0) wait.
        with tc.tile_pool(name="psum_w", bufs=1, space="PSUM") as pwarm:
            wm = pwarm.tile([CIJ, CIJ], dt.float32)
            nc.tensor.matmul(
                wm[:], wce[:, 0:CIJ], wce[:, 0:CIJ], start=True, stop=True,
                skip_group_check=True,
            )

        with (
            tc.tile_pool(name="psum_y", bufs=1, space="PSUM") as pyp,
            tc.tile_pool(name="psum_a", bufs=2, space="PSUM") as ppa,
            tc.tile_pool(name="psum_b", bufs=2, space="PSUM") as ppb,
        ):
            # y-partial accumulators: [128b x 512eo] + [128b x 256eo] per
            # b-half; each tile is a full PSUM bank.
            py = [
                [
                    pyp.tile([128, 512], dt.float32, tag=f"py{bh}0", name=f"py{bh}0"),
                    pyp.tile([128, 512], dt.float32, tag=f"py{bh}1", name=f"py{bh}1"),
                ]
                for bh in range(2)
            ]

            w2tiles = {}

            def w2_block(p, wl, base):
                pa = ppa.tile([CIJ, 512], dt.float32)
                pb = ppb.tile([CIJ, 512], dt.float32)
                for ech in range(NECH):
                    lhsT = wce[:, ech * CIJ : (ech + 1) * CIJ]
                    nc.tensor.matmul(
                        pa[:, 0:512],
                        lhsT,
                        wl[:, base + ech * E : base + ech * E + 512],
                        start=(ech == 0),
                        stop=(ech == NECH - 1),
                        skip_group_check=True,
                    )
                    nc.tensor.matmul(
                        pb[:, 0:256],
                        lhsT,
                        wl[:, base + ech * E + 512 : base + ech * E + 768],
                        start=(ech == 0),
                        stop=(ech == NECH - 1),
                        skip_group_check=True,
                    )
                w2t = w2_pool.tile([CIJ, E], dt.bfloat16)
                w2tiles[p] = w2t
                # PSUM f32 -> SBUF bf16 cast-copies; alternate engines by p.
                if p % 2 == 0:
                    nc.vector.tensor_copy(w2t[:, 0:512], pa[:, 0:512])
                    nc.vector.tensor_copy(w2t[:, 512:768], pb[:, 0:256])
                else:
                    nc.scalar.copy(w2t[:, 0:512], pa[:, 0:512])
                    nc.scalar.copy(w2t[:, 512:768], pb[:, 0:256])

            def final_block(p):
                w2t = w2tiles.pop(p)
                for bh in range(2):
                    lhsT = xpp[:, p * B + bh * 128 : p * B + bh * 128 + 128]
                    nc.tensor.matmul(
                        py[bh][0][:, 0:512],
                        lhsT,
                        w2t[:, 0:512],
                        start=(p == 0),
                        stop=(p == PL - 1),
                        skip_group_check=True,
                    )
                    nc.tensor.matmul(
                        py[bh][1][:, 0:256],
                        lhsT,
                        w2t[:, 512:768],
                        start=(p == 0),
                        stop=(p == PL - 1),
                        skip_group_check=True,
                    )

            # wlp arrives in 2-patch DMAs (18432B per-partition lines) that
            # alternate between the gpsimd and sync software queues; the
            # final(p) block is emitted one patch behind W2(p) so the PE
            # never stalls on the PSUM->SBUF copy round-trip of W2(p).
            for pt in range(PL // 2):
                wl = wlp_pool.tile([128, 2 * PCOL], dt.bfloat16)
                eng = nc.gpsimd if pt % 2 == 0 else nc.sync
                eng.dma_start(
                    out=wl[:], in_=wlp_d[:, pt * 2 * PCOL : (pt + 1) * 2 * PCOL]
                )
                for j in range(2):
                    p = 2 * pt + j
                    w2_block(p, wl, j * PCOL)
                    if p >= 1:
                        final_block(p - 1)
            final_block(PL - 1)

            for bh in range(2):
                ob = out_pool.tile([128, E], dt.float32, tag=f"ob{bh}")
                nc.vector.tensor_copy(ob[:, 0:512], py[bh][0][:, 0:512])
                nc.scalar.copy(ob[:, 512:768], py[bh][1][:, 0:256])
                nc.sync.dma_start(
                    out=out_d[bh * 128 : (bh + 1) * 128, :], in_=ob[:]
                )
    _split_extra_waits(nc)
    return nc


def _split_extra_waits(nc):
    """Walrus encodes at most one semaphore wait on regular engine
    instructions (Matmult, DMACopy, ...). When Tile attaches more (e.g.
    slot-recycle release + data-ready on different procs), split the extras
    onto InstEventSemaphore instructions inserted immediately before the
    instruction on the same engine queue -- semantically identical to the
    multi-wait (the engine blocks at the same point for all of them)."""
    import bass_rust
    import concourse.mybir as mybir

    keep_multi = {"InstEventSemaphore", "InstUnconditionalBranch"}
    n_split = 0
    for fn in nc.m.functions:
        for bb in fn.blocks:
            out = []
            changed = False
            for ins in bb.instructions:
                si = ins.sync_info
                if (
                    si is not None
                    and len(si.on_wait) > 1
                    and type(ins).__name__ not in keep_multi
                ):
                    waits = list(si.on_wait)
                    for w in waits[:-1]:
                        ev = mybir.InstEventSemaphore(
                            name=f"W-split-{n_split}", ins=[], outs=[]
                        )
                        n_split += 1
                        ev.engine = ins.engine
                        ev.sync_info = bass_rust.SyncInfo(on_wait=[w], on_update=[])
                        out.append(ev)
                    ins.sync_info = bass_rust.SyncInfo(
                        on_wait=[waits[-1]], on_update=list(si.on_update)
                    )
                    changed = True
                out.append(ins)
            if changed:
                bb.instructions = out
    return n_split


def _prep_inputs(x, wconv, bconv, wlin):
    bf16 = ml_dtypes.bfloat16
    x = np.ascontiguousarray(np.asarray(x, dtype=np.float32))
    wconv = np.asarray(wconv, dtype=np.float32)
    bconv = np.asarray(bconv, dtype=np.float32)
    wlin = np.asarray(wlin, dtype=np.float32)

    # im2col: xpa[(c,i,j), b, p] = x[b, c, 4hp+i, 4wp+j], p = hp*16+wp;
    # row 48 = ones (bias row). Pure index remap, zero FLOPs.
    xp = x.reshape(B, C, Hp, P, Wp, P).transpose(1, 3, 5, 0, 2, 4)
    xpa = np.empty((CIJ, B, NP), np.float32)
    xpa[:48] = xp.reshape(48, B, NP)
    xpa[48] = 1.0

    # wce[e_r, ech, cij] = wcaT[ech*128+e_r, cij]; wca row 48 = bconv.
    wca = np.empty((CIJ, E), np.float32)
    wca[:48] = wconv.reshape(E, 48).T
    wca[48] = bconv
    wce = np.ascontiguousarray(
        wca.T.reshape(NECH, 128, CIJ).transpose(1, 0, 2).reshape(128, NECH * CIJ)
    ).astype(bf16)

    wlinR = wlin.reshape(E, E, NP)  # [eo, e, p]
    in_maps = []
    for c in range(NCORES):
        ps = c * PL
        # wlp[e_r, p*4608 + ech*768 + eo] = wlin[eo, (ech*128+e_r)*256 + p]
        wlp = (
            wlinR[:, :, ps : ps + PL]
            .transpose(1, 2, 0)                 # [e, p, eo]
            .reshape(NECH, 128, PL, E)
            .transpose(1, 2, 0, 3)              # [e_r, p, ech, eo]
            .reshape(128, PL * PCOL)
            .astype(bf16)
        )
        xpp = (
            xpa[:, :, ps : ps + PL]
            .transpose(0, 2, 1)                 # [cij, p, b]
            .reshape(CIJ, PL * B)
            .astype(bf16)
        )
        in_maps.append({"wce": wce, "xpp": xpp, "wlp": wlp})
    return in_maps


def _patch_ldw_opt():
    """walrus is invoked with --enable-ldw-opt=false (hardcoded); enabling it
    lets codegen elide redundant LDWEIGHTS. Rewrite the flag on the way in."""
    from concourse import bass_utils as _bu

    if getattr(_bu, "_ldw_opt_patched", False):
        return
    _orig = _bu.run_command

    def _patched(cmd, **kw):
        if isinstance(cmd, list):
            cmd = [
                "--enable-ldw-opt=true" if c == "--enable-ldw-opt=false" else c
                for c in cmd
            ]
        return _orig(cmd, **kw)

    _bu.run_command = _patched
    _bu._ldw_opt_patched = True


def _run(x, wconv, bconv, wlin, blin, trace=False, **trace_kwargs):
    from concourse.bass_utils import run_bass_kernel_spmd

    if "nc" not in _CACHE:
        _CACHE["nc"] = _build_bass()
    in_maps = _prep_inputs(x, wconv, bconv, wlin)
    res = run_bass_kernel_spmd(
        _CACHE["nc"], in_maps, core_ids=list(range(NCORES)), trace=trace,
        **trace_kwargs,
    )
    acc = np.zeros((B, E), np.float64)
    for r in res.results:
        acc += r["y"]
    y = (acc + np.asarray(blin, dtype=np.float64)[None, :]).astype(np.float32)
    return y, res


def kernel(x, wconv, bconv, wlin, blin, patch_size):
    assert int(patch_size) == P
    y, _ = _run(x, wconv, bconv, wlin, blin, trace=False)
    return y


# revision 12
# speedup vs baseline: 1.1579x; 1.1579x over previous
"""Trainium2 Bass kernel: patch-conv (Conv2d C3->E768, k4 s4) + giant linear.

y[b, eo] = sum_K flat[b, K] * wlin[eo, K] + blin[eo],
flat[b, e*256+p] = conv[b, e, p] (+ bconv[e]), K = 196608.

Reassociated algorithm (matrix-chain reordering, all FLOPs on device):
    y[b,eo] = sum_{p,cij} xp[cij,p,b] * W2[p,cij,eo]
    W2[p,cij,eo] = sum_e wca[cij,e] * wlin[eo, e*256+p]
where xp is the im2col of x (pure index remap, row 48 = ones) and wca is
wconv reshaped [49, E] (row 48 = bconv). This computes the identical
function with 19.7 GFLOP instead of 82 GFLOP, and reads wlin exactly once.

Sharding (8 cores): shard the patch index p (32 patches/core). Each core:
  - reads its wlin slice re-laid-out on host as wlp[e_r, (p, ech, eo)] bf16
    (37.75 MB -- the DMA roofline term),
  - W2-mm: for each p: 6 e-chunks x (N=512 + N=256) matmuls, stationary
    wce[e_chunk] = wcaT slice [128,49], accumulate PSUM [49,768],
  - copies W2_p to SBUF bf16 (vector/scalar alternating),
  - final-mm: stationary xpp[:, p, b-half] [49,128], moving W2_p, PSUM
    accumulates y-partial [128b, 512|256 eo] over the 32 p's.
  - Host: sum the 8 partial y's, add blin.
All PSUM tiles are bank-sized (2048B) so accumulation groups never cross
a PSUM bank: 4 banks y-accum + 2x2 banks W2-accum = 8.
"""

import numpy as np
import ml_dtypes

B, C, H, W = 256, 3, 64, 64
P, Hp, Wp, NP = 4, 16, 16, 256
E = 768
CIJ = 49                  # 48 conv taps + 1 bias row
NCORES = 8
PL = NP // NCORES         # 32 patches per core
NECH = E // 128           # 6 e-chunks in the W2 contraction
PCOL = NECH * E           # 4608 wlp columns per patch

_CACHE = {}


def _build_bass():
    import concourse.bass as bass
    import concourse.mybir as mybir
    import concourse.tile as tile
    from contextlib import ExitStack

    dt = mybir.dt
    nc = bass.Bass()
    wce_d = nc.dram_tensor("wce", [128, NECH * CIJ], dt.bfloat16, kind="ExternalInput")
    xpp_d = nc.dram_tensor("xpp", [CIJ, PL * B], dt.bfloat16, kind="ExternalInput")
    wlp_d = nc.dram_tensor("wlp", [128, PL * PCOL], dt.bfloat16, kind="ExternalInput")
    out_d = nc.dram_tensor("y", [B, E], dt.float32, kind="ExternalOutput")

    with tile.TileContext(nc) as tc, ExitStack() as ctx:
        singles = ctx.enter_context(tc.tile_pool(name="singles", bufs=1))
        # wce + xpp ride the gpsimd queue; the wlp stream owns the sync
        # (HWDGE) queue exclusively -- HWDGE avoids the SWDGE SBUF
        # descriptor-ring contention that slows SDMA engines 7/15.
        wce = singles.tile([128, NECH * CIJ], dt.bfloat16)
        nc.gpsimd.dma_start(out=wce[:], in_=wce_d[:])
        xpp = singles.tile([CIJ, PL * B], dt.bfloat16)
        nc.gpsimd.dma_start(out=xpp[:, 0 : PL * B // 2], in_=xpp_d[:, 0 : PL * B // 2])
        nc.gpsimd.dma_start(out=xpp[:, PL * B // 2 :], in_=xpp_d[:, PL * B // 2 :])

        # W2 staging ring: one [49, 768] bf16 tile per patch, recycled.
        w2_pool = ctx.enter_context(tc.tile_pool(name="w2ring", bufs=6))
        wlp_pool = ctx.enter_context(tc.tile_pool(name="wlp", bufs=8))
        out_pool = ctx.enter_context(tc.tile_pool(name="out_sb", bufs=1))

        # Warmup: absorb the wce DMA-ready wait on a throwaway matmul so the
        # first real W2 matmul carries only the wlp(0) wait.
        with tc.tile_pool(name="psum_w", bufs=1, space="PSUM") as pwarm:
            wm = pwarm.tile([CIJ, CIJ], dt.float32)
            nc.tensor.matmul(
                wm[:], wce[:, 0:CIJ], wce[:, 0:CIJ], start=True, stop=True,
                skip_group_check=True,
            )

        with (
            tc.tile_pool(name="psum_y", bufs=1, space="PSUM") as pyp,
            tc.tile_pool(name="psum_a", bufs=2, space="PSUM") as ppa,
            tc.tile_pool(name="psum_b", bufs=2, space="PSUM") as ppb,
        ):
            # y-partial accumulators: [128b x 512eo] + [128b x 256eo] per
            # b-half; each tile is a full PSUM bank.
            py = [
                [
                    pyp.tile([128, 512], dt.float32, tag=f"py{bh}0", name=f"py{bh}0"),
                    pyp.tile([128, 512], dt.float32, tag=f"py{bh}1", name=f"py{bh}1"),
                ]
                for bh in range(2)
            ]

            w2tiles = {}

            def w2_block(p, wl, base):
                pa = ppa.tile([CIJ, 512], dt.float32)
                pb = ppb.tile([CIJ, 512], dt.float32)
                for ech in range(NECH):
                    lhsT = wce[:, ech * CIJ : (ech + 1) * CIJ]
                    nc.tensor.matmul(
                        pa[:, 0:512],
                        lhsT,
                        wl[:, base + ech * E : base + ech * E + 512],
                        start=(ech == 0),
                        stop=(ech == NECH - 1),
                        skip_group_check=True,
                    )
                    nc.tensor.matmul(
                        pb[:, 0:256],
                        lhsT,
                        wl[:, base + ech * E + 512 : base + ech * E + 768],
                        start=(ech == 0),
                        stop=(ech == NECH - 1),
                        skip_group_check=True,
                    )
                w2t = w2_pool.tile([CIJ, E], dt.bfloat16)
                w2tiles[p] = w2t
                # PSUM f32 -> SBUF bf16 cast-copies; alternate engines by p.
                if p % 2 == 0:
                    nc.vector.tensor_copy(w2t[:, 0:512], pa[:, 0:512])
                    nc.vector.tensor_copy(w2t[:, 512:768], pb[:, 0:256])
                else:
                    nc.scalar.copy(w2t[:, 0:512], pa[:, 0:512])
                    nc.scalar.copy(w2t[:, 512:768], pb[:, 0:256])

            def final_block(p):
                w2t = w2tiles.pop(p)
                for bh in range(2):
                    lhsT = xpp[:, p * B + bh * 128 : p * B + bh * 128 + 128]
                    nc.tensor.matmul(
                        py[bh][0][:, 0:512],
                        lhsT,
                        w2t[:, 0:512],
                        start=(p == 0),
                        stop=(p == PL - 1),
                        skip_group_check=True,
                    )
                    nc.tensor.matmul(
                        py[bh][1][:, 0:256],
                        lhsT,
                        w2t[:, 512:768],
                        start=(p == 0),
                        stop=(p == PL - 1),
                        skip_group_check=True,
                    )

            # wlp streams as one sequential per-patch DMA chain on the sync
            # HWDGE queue (9216B per-partition lines, one packet each); the
            # final(p) block is emitted one patch behind W2(p) so the PE
            # never stalls on the PSUM->SBUF copy round-trip of W2(p).
            for p in range(PL):
                wl = wlp_pool.tile([128, PCOL], dt.bfloat16)
                nc.sync.dma_start(
                    out=wl[:], in_=wlp_d[:, p * PCOL : (p + 1) * PCOL]
                )
                w2_block(p, wl, 0)
                if p >= 1:
                    final_block(p - 1)
            final_block(PL - 1)

            for bh in range(2):
                ob = out_pool.tile([128, E], dt.float32, tag=f"ob{bh}")
                nc.vector.tensor_copy(ob[:, 0:512], py[bh][0][:, 0:512])
                nc.scalar.copy(ob[:, 512:768], py[bh][1][:, 0:256])
                nc.gpsimd.dma_start(
                    out=out_d[bh * 128 : (bh + 1) * 128, :], in_=ob[:]
                )
    _split_extra_waits(nc)
    return nc


def _split_extra_waits(nc):
    """Walrus encodes at most one semaphore wait on regular engine
    instructions (Matmult, DMACopy, ...). When Tile attaches more (e.g.
    slot-recycle release + data-ready on different procs), split the extras
    onto InstEventSemaphore instructions inserted immediately before the
    instruction on the same engine queue -- semantically identical to the
    multi-wait (the engine blocks at the same point for all of them)."""
    import bass_rust
    import concourse.mybir as mybir

    keep_multi = {"InstEventSemaphore", "InstUnconditionalBranch"}
    n_split = 0
    for fn in nc.m.functions:
        for bb in fn.blocks:
            out = []
            changed = False
            for ins in bb.instructions:
                si = ins.sync_info
                if (
                    si is not None
                    and len(si.on_wait) > 1
                    and type(ins).__name__ not in keep_multi
                ):
                    waits = list(si.on_wait)
                    for w in waits[:-1]:
                        ev = mybir.InstEventSemaphore(
                            name=f"W-split-{n_split}", ins=[], outs=[]
                        )
                        n_split += 1
                        ev.engine = ins.engine
                        ev.sync_info = bass_rust.SyncInfo(on_wait=[w], on_update=[])
                        out.append(ev)
                    ins.sync_info = bass_rust.SyncInfo(
                        on_wait=[waits[-1]], on_update=list(si.on_update)
                    )
                    changed = True
                out.append(ins)
            if changed:
                bb.instructions = out
    return n_split


def _prep_inputs(x, wconv, bconv, wlin):
    bf16 = ml_dtypes.bfloat16
    x = np.ascontiguousarray(np.asarray(x, dtype=np.float32))
    wconv = np.asarray(wconv, dtype=np.float32)
    bconv = np.asarray(bconv, dtype=np.float32)
    wlin = np.asarray(wlin, dtype=np.float32)

    # im2col: xpa[(c,i,j), b, p] = x[b, c, 4hp+i, 4wp+j], p = hp*16+wp;
    # row 48 = ones (bias row). Pure index remap, zero FLOPs.
    xp = x.reshape(B, C, Hp, P, Wp, P).transpose(1, 3, 5, 0, 2, 4)
    xpa = np.empty((CIJ, B, NP), np.float32)
    xpa[:48] = xp.reshape(48, B, NP)
    xpa[48] = 1.0

    # wce[e_r, ech, cij] = wcaT[ech*128+e_r, cij]; wca row 48 = bconv.
    wca = np.empty((CIJ, E), np.float32)
    wca[:48] = wconv.reshape(E, 48).T
    wca[48] = bconv
    wce = np.ascontiguousarray(
        wca.T.reshape(NECH, 128, CIJ).transpose(1, 0, 2).reshape(128, NECH * CIJ)
    ).astype(bf16)

    wlinR = wlin.reshape(E, E, NP)  # [eo, e, p]
    in_maps = []
    for c in range(NCORES):
        ps = c * PL
        # wlp[e_r, p*4608 + ech*768 + eo] = wlin[eo, (ech*128+e_r)*256 + p]
        wlp = (
            wlinR[:, :, ps : ps + PL]
            .transpose(1, 2, 0)                 # [e, p, eo]
            .reshape(NECH, 128, PL, E)
            .transpose(1, 2, 0, 3)              # [e_r, p, ech, eo]
            .reshape(128, PL * PCOL)
            .astype(bf16)
        )
        xpp = (
            xpa[:, :, ps : ps + PL]
            .transpose(0, 2, 1)                 # [cij, p, b]
            .reshape(CIJ, PL * B)
            .astype(bf16)
        )
        in_maps.append({"wce": wce, "xpp": xpp, "wlp": wlp})
    return in_maps


def _patch_ldw_opt():
    """walrus is invoked with --enable-ldw-opt=false (hardcoded); enabling it
    lets codegen elide redundant LDWEIGHTS. Rewrite the flag on the way in."""
    from concourse import bass_utils as _bu

    if getattr(_bu, "_ldw_opt_patched", False):
        return
    _orig = _bu.run_command

    def _patched(cmd, **kw):
        if isinstance(cmd, list):
            cmd = [
                "--enable-ldw-opt=true" if c == "--enable-ldw-opt=false" else c
                for c in cmd
            ]
        return _orig(cmd, **kw)

    _bu.run_command = _patched
    _bu._ldw_opt_patched = True


def _run(x, wconv, bconv, wlin, blin, trace=False, **trace_kwargs):
    from concourse.bass_utils import run_bass_kernel_spmd

    if "nc" not in _CACHE:
        _CACHE["nc"] = _build_bass()
    in_maps = _prep_inputs(x, wconv, bconv, wlin)
    res = run_bass_kernel_spmd(
        _CACHE["nc"], in_maps, core_ids=list(range(NCORES)), trace=trace,
        **trace_kwargs,
    )
    acc = np.zeros((B, E), np.float64)
    for r in res.results:
        acc += r["y"]
    y = (acc + np.asarray(blin, dtype=np.float64)[None, :]).astype(np.float32)
    return y, res


def kernel(x, wconv, bconv, wlin, blin, patch_size):
    assert int(patch_size) == P
    y, _ = _run(x, wconv, bconv, wlin, blin, trace=False)
    return y


# revision 18
# speedup vs baseline: 1.2260x; 1.0588x over previous
"""Trainium2 Bass kernel: patch-conv (Conv2d C3->E768, k4 s4) + giant linear.

y[b, eo] = sum_K flat[b, K] * wlin[eo, K] + blin[eo],
flat[b, e*256+p] = conv[b, e, p] (+ bconv[e]), K = 196608.

Reassociated algorithm (matrix-chain reordering, all FLOPs on device):
    y[b,eo] = sum_{p,cij} xp[cij,p,b] * W2[p,cij,eo]
    W2[p,cij,eo] = sum_e wca[cij,e] * wlin[eo, e*256+p]
where xp is the im2col of x (pure index remap, row 48 = ones) and wca is
wconv reshaped [49, E] (row 48 = bconv). This computes the identical
function with 19.7 GFLOP instead of 82 GFLOP, and reads wlin exactly once.

Sharding (8 cores): shard the patch index p (32 patches/core). Each core:
  - reads its wlin slice re-laid-out on host as wlp[e_r, (p, ech, eo)] bf16
    (37.75 MB -- the DMA roofline term),
  - W2-mm: for each p: 6 e-chunks x (N=512 + N=256) matmuls, stationary
    wce[e_chunk] = wcaT slice [128,49], accumulate PSUM [49,768],
  - copies W2_p to SBUF bf16 (vector/scalar alternating),
  - final-mm: stationary xpp[:, p, b-half] [49,128], moving W2_p, PSUM
    accumulates y-partial [128b, 512|256 eo] over the 32 p's.
  - Host: sum the 8 partial y's, add blin.
All PSUM tiles are bank-sized (2048B) so accumulation groups never cross
a PSUM bank: 4 banks y-accum + 2x2 banks W2-accum = 8.
"""

import numpy as np
import ml_dtypes

B, C, H, W = 256, 3, 64, 64
P, Hp, Wp, NP = 4, 16, 16, 256
E = 768
CIJ = 49                  # 48 conv taps + 1 bias row
NCORES = 8
PL = NP // NCORES         # 32 patches per core
NECH = E // 128           # 6 e-chunks in the W2 contraction
PCOL = NECH * E           # 4608 wlp columns per patch

_CACHE = {}


def _build_bass():
    import concourse.bass as bass
    import concourse.mybir as mybir
    import concourse.tile as tile
    from contextlib import ExitStack

    dt = mybir.dt
    nc = bass.Bass()
    wce_d = nc.dram_tensor("wce", [128, NECH * CIJ], dt.bfloat16, kind="ExternalInput")
    xpp_d = nc.dram_tensor("xpp", [CIJ, PL * B], dt.bfloat16, kind="ExternalInput")
    wlp_d = nc.dram_tensor("wlp", [128, PL * PCOL], dt.bfloat16, kind="ExternalInput")
    out_d = nc.dram_tensor("y", [B, E], dt.float32, kind="ExternalOutput")

    with tile.TileContext(nc) as tc, ExitStack() as ctx:
        singles = ctx.enter_context(tc.tile_pool(name="singles", bufs=1))
        # wce + xpp ride the gpsimd queue; the wlp stream owns the sync
        # (HWDGE) queue exclusively -- HWDGE avoids the SWDGE SBUF
        # descriptor-ring contention that slows SDMA engines 7/15.
        wce = singles.tile([128, NECH * CIJ], dt.bfloat16)
        nc.gpsimd.dma_start(out=wce[:], in_=wce_d[:])
        xpp = singles.tile([CIJ, PL * B], dt.bfloat16)
        nc.gpsimd.dma_start(out=xpp[:, 0 : PL * B // 2], in_=xpp_d[:, 0 : PL * B // 2])
        nc.gpsimd.dma_start(out=xpp[:, PL * B // 2 :], in_=xpp_d[:, PL * B // 2 :])

        # W2 staging ring: one [49, 768] bf16 tile per patch, recycled.
        w2_pool = ctx.enter_context(tc.tile_pool(name="w2ring", bufs=6))
        wlp_pool = ctx.enter_context(tc.tile_pool(name="wlp", bufs=4))
        out_pool = ctx.enter_context(tc.tile_pool(name="out_sb", bufs=1))

        # Warmup: absorb the wce DMA-ready wait on a throwaway matmul so the
        # first real W2 matmul carries only the wlp(0) wait.
        with tc.tile_pool(name="psum_w", bufs=1, space="PSUM") as pwarm:
            wm = pwarm.tile([CIJ, CIJ], dt.float32)
            nc.tensor.matmul(
                wm[:], wce[:, 0:CIJ], wce[:, 0:CIJ], start=True, stop=True,
                skip_group_check=True,
            )

        with (
            tc.tile_pool(name="psum_y", bufs=1, space="PSUM") as pyp,
            tc.tile_pool(name="psum_w2", bufs=1, space="PSUM") as ppw,
        ):
            # y-partial accumulators: [128b x 512eo] + [128b x 256eo] per
            # b-half; each tile is a full PSUM bank.
            py = [
                [
                    pyp.tile([128, 512], dt.float32, tag=f"py{bh}0", name=f"py{bh}0"),
                    pyp.tile([128, 512], dt.float32, tag=f"py{bh}1", name=f"py{bh}1"),
                ]
                for bh in range(2)
            ]

            w2tiles = {}
            # Four bank-sized W2 accumulators (two patches in flight); bufs=1
            # per tag: pt+1's start=True reuse waits only on pt's copy.
            pw = [
                [
                    ppw.tile([CIJ, 512], dt.float32, tag=f"pa{j}", name=f"pa{j}"),
                    ppw.tile([CIJ, 512], dt.float32, tag=f"pb{j}", name=f"pb{j}"),
                ]
                for j in range(2)
            ]

            def w2_pair_block(pt, wl):
                # One LDWEIGHTS per (pt, ech): the four matmuls (2 patches x
                # 2 eo-slices) share the stationary wce e-chunk, so walrus
                # ldw-opt elides the redundant loads.
                for ech in range(NECH):
                    lhsT = wce[:, ech * CIJ : (ech + 1) * CIJ]
                    for j in range(2):
                        base = j * PCOL
                        nc.tensor.matmul(
                            pw[j][0][:, 0:512],
                            lhsT,
                            wl[:, base + ech * E : base + ech * E + 512],
                            start=(ech == 0),
                            stop=(ech == NECH - 1),
                            skip_group_check=True,
                        )
                        nc.tensor.matmul(
                            pw[j][1][:, 0:256],
                            lhsT,
                            wl[:, base + ech * E + 512 : base + ech * E + 768],
                            start=(ech == 0),
                            stop=(ech == NECH - 1),
                            skip_group_check=True,
                        )
                # PSUM f32 -> SBUF bf16 cast-copies; one engine per patch.
                for j in range(2):
                    p = 2 * pt + j
                    w2t = w2_pool.tile([CIJ, E], dt.bfloat16)
                    w2tiles[p] = w2t
                    if j == 0:
                        nc.vector.tensor_copy(w2t[:, 0:512], pw[j][0][:, 0:512])
                        nc.vector.tensor_copy(w2t[:, 512:768], pw[j][1][:, 0:256])
                    else:
                        nc.scalar.copy(w2t[:, 0:512], pw[j][0][:, 0:512])
                        nc.scalar.copy(w2t[:, 512:768], pw[j][1][:, 0:256])

            def final_block(p):
                w2t = w2tiles.pop(p)
                for bh in range(2):
                    lhsT = xpp[:, p * B + bh * 128 : p * B + bh * 128 + 128]
                    nc.tensor.matmul(
                        py[bh][0][:, 0:512],
                        lhsT,
                        w2t[:, 0:512],
                        start=(p == 0),
                        stop=(p == PL - 1),
                        skip_group_check=True,
                    )
                    nc.tensor.matmul(
                        py[bh][1][:, 0:256],
                        lhsT,
                        w2t[:, 512:768],
                        start=(p == 0),
                        stop=(p == PL - 1),
                        skip_group_check=True,
                    )

            # wlp streams as one sequential 2-patch DMA chain on the sync
            # HWDGE queue (18432B per-partition lines, one packet each); the
            # final blocks run one patch-pair behind W2 so the PE never
            # stalls on the PSUM->SBUF copy round-trip.
            for pt in range(PL // 2):
                wl = wlp_pool.tile([128, 2 * PCOL], dt.bfloat16)
                nc.sync.dma_start(
                    out=wl[:], in_=wlp_d[:, pt * 2 * PCOL : (pt + 1) * 2 * PCOL]
                )
                w2_pair_block(pt, wl)
                if pt >= 1:
                    final_block(2 * pt - 2)
                    final_block(2 * pt - 1)
            final_block(PL - 2)
            final_block(PL - 1)

            for bh in range(2):
                ob = out_pool.tile([128, E], dt.float32, tag=f"ob{bh}")
                nc.vector.tensor_copy(ob[:, 0:512], py[bh][0][:, 0:512])
                nc.scalar.copy(ob[:, 512:768], py[bh][1][:, 0:256])
                nc.gpsimd.dma_start(
                    out=out_d[bh * 128 : (bh + 1) * 128, :], in_=ob[:]
                )
    _split_extra_waits(nc)
    return nc


def _split_extra_waits(nc):
    """Walrus encodes at most one semaphore wait on regular engine
    instructions (Matmult, DMACopy, ...). When Tile attaches more (e.g.
    slot-recycle release + data-ready on different procs), split the extras
    onto InstEventSemaphore instructions inserted immediately before the
    instruction on the same engine queue -- semantically identical to the
    multi-wait (the engine blocks at the same point for all of them)."""
    import bass_rust
    import concourse.mybir as mybir

    keep_multi = {"InstEventSemaphore", "InstUnconditionalBranch"}
    n_split = 0
    for fn in nc.m.functions:
        for bb in fn.blocks:
            out = []
            changed = False
            for ins in bb.instructions:
                si = ins.sync_info
                if (
                    si is not None
                    and len(si.on_wait) > 1
                    and type(ins).__name__ not in keep_multi
                ):
                    waits = list(si.on_wait)
                    for w in waits[:-1]:
                        ev = mybir.InstEventSemaphore(
                            name=f"W-split-{n_split}", ins=[], outs=[]
                        )
                        n_split += 1
                        ev.engine = ins.engine
                        ev.sync_info = bass_rust.SyncInfo(on_wait=[w], on_update=[])
                        out.append(ev)
                    ins.sync_info = bass_rust.SyncInfo(
                        on_wait=[waits[-1]], on_update=list(si.on_update)
                    )
                    changed = True
                out.append(ins)
            if changed:
                bb.instructions = out
    return n_split


def _prep_inputs(x, wconv, bconv, wlin):
    bf16 = ml_dtypes.bfloat16
    x = np.ascontiguousarray(np.asarray(x, dtype=np.float32))
    wconv = np.asarray(wconv, dtype=np.float32)
    bconv = np.asarray(bconv, dtype=np.float32)
    wlin = np.asarray(wlin, dtype=np.float32)

    # im2col: xpa[(c,i,j), b, p] = x[b, c, 4hp+i, 4wp+j], p = hp*16+wp;
    # row 48 = ones (bias row). Pure index remap, zero FLOPs.
    xp = x.reshape(B, C, Hp, P, Wp, P).transpose(1, 3, 5, 0, 2, 4)
    xpa = np.empty((CIJ, B, NP), np.float32)
    xpa[:48] = xp.reshape(48, B, NP)
    xpa[48] = 1.0

    # wce[e_r, ech, cij] = wcaT[ech*128+e_r, cij]; wca row 48 = bconv.
    wca = np.empty((CIJ, E), np.float32)
    wca[:48] = wconv.reshape(E, 48).T
    wca[48] = bconv
    wce = np.ascontiguousarray(
        wca.T.reshape(NECH, 128, CIJ).transpose(1, 0, 2).reshape(128, NECH * CIJ)
    ).astype(bf16)

    wlinR = wlin.reshape(E, E, NP)  # [eo, e, p]
    in_maps = []
    for c in range(NCORES):
        ps = c * PL
        # wlp[e_r, p*4608 + ech*768 + eo] = wlin[eo, (ech*128+e_r)*256 + p]
        wlp = (
            wlinR[:, :, ps : ps + PL]
            .transpose(1, 2, 0)                 # [e, p, eo]
            .reshape(NECH, 128, PL, E)
            .transpose(1, 2, 0, 3)              # [e_r, p, ech, eo]
            .reshape(128, PL * PCOL)
            .astype(bf16)
        )
        xpp = (
            xpa[:, :, ps : ps + PL]
            .transpose(0, 2, 1)                 # [cij, p, b]
            .reshape(CIJ, PL * B)
            .astype(bf16)
        )
        in_maps.append({"wce": wce, "xpp": xpp, "wlp": wlp})
    return in_maps


def _patch_ldw_opt():
    """walrus is invoked with --enable-ldw-opt=false (hardcoded); enabling it
    lets codegen elide redundant LDWEIGHTS. Rewrite the flag on the way in."""
    from concourse import bass_utils as _bu

    if getattr(_bu, "_ldw_opt_patched", False):
        return
    _orig = _bu.run_command

    def _patched(cmd, **kw):
        if isinstance(cmd, list):
            cmd = [
                "--enable-ldw-opt=true" if c == "--enable-ldw-opt=false" else c
                for c in cmd
            ]
        return _orig(cmd, **kw)

    _bu.run_command = _patched
    _bu._ldw_opt_patched = True


def _run(x, wconv, bconv, wlin, blin, trace=False, **trace_kwargs):
    from concourse.bass_utils import run_bass_kernel_spmd

    if "nc" not in _CACHE:
        _CACHE["nc"] = _build_bass()
    in_maps = _prep_inputs(x, wconv, bconv, wlin)
    res = run_bass_kernel_spmd(
        _CACHE["nc"], in_maps, core_ids=list(range(NCORES)), trace=trace,
        **trace_kwargs,
    )
    acc = np.zeros((B, E), np.float64)
    for r in res.results:
        acc += r["y"]
    y = (acc + np.asarray(blin, dtype=np.float64)[None, :]).astype(np.float32)
    return y, res


def kernel(x, wconv, bconv, wlin, blin, patch_size):
    assert int(patch_size) == P
    y, _ = _run(x, wconv, bconv, wlin, blin, trace=False)
    return y


# revision 19
# speedup vs baseline: 1.2406x; 1.0119x over previous
"""Trainium2 Bass kernel: patch-conv (Conv2d C3->E768, k4 s4) + giant linear.

y[b, eo] = sum_K flat[b, K] * wlin[eo, K] + blin[eo],
flat[b, e*256+p] = conv[b, e, p] (+ bconv[e]), K = 196608.

Reassociated algorithm (matrix-chain reordering, all FLOPs on device):
    y[b,eo] = sum_{p,cij} xp[cij,p,b] * W2[p,cij,eo]
    W2[p,cij,eo] = sum_e wca[cij,e] * wlin[eo, e*256+p]
where xp is the im2col of x (pure index remap, row 48 = ones) and wca is
wconv reshaped [49, E] (row 48 = bconv). This computes the identical
function with 19.7 GFLOP instead of 82 GFLOP, and reads wlin exactly once.

Sharding (8 cores): shard the patch index p (32 patches/core). Each core:
  - reads its wlin slice re-laid-out on host as wlp[e_r, (p, ech, eo)] bf16
    (37.75 MB -- the DMA roofline term),
  - W2-mm: for each p: 6 e-chunks x (N=512 + N=256) matmuls, stationary
    wce[e_chunk] = wcaT slice [128,49], accumulate PSUM [49,768],
  - copies W2_p to SBUF bf16 (vector/scalar alternating),
  - final-mm: stationary xpp[:, p, b-half] [49,128], moving W2_p, PSUM
    accumulates y-partial [128b, 512|256 eo] over the 32 p's.
  - Host: sum the 8 partial y's, add blin.
All PSUM tiles are bank-sized (2048B) so accumulation groups never cross
a PSUM bank: 4 banks y-accum + 2x2 banks W2-accum = 8.
"""

import numpy as np
import ml_dtypes

B, C, H, W = 256, 3, 64, 64
P, Hp, Wp, NP = 4, 16, 16, 256
E = 768
CIJ = 49                  # 48 conv taps + 1 bias row
NCORES = 8
PL = NP // NCORES         # 32 patches per core
NECH = E // 128           # 6 e-chunks in the W2 contraction
PCOL = NECH * E           # 4608 wlp columns per patch

_CACHE = {}


def _build_bass():
    import concourse.bass as bass
    import concourse.mybir as mybir
    import concourse.tile as tile
    from contextlib import ExitStack

    dt = mybir.dt
    nc = bass.Bass()
    wce_d = nc.dram_tensor("wce", [128, NECH * CIJ], dt.bfloat16, kind="ExternalInput")
    xpp_d = nc.dram_tensor("xpp", [CIJ, PL * B], dt.bfloat16, kind="ExternalInput")
    wlp_d = nc.dram_tensor("wlp", [128, PL * PCOL], dt.bfloat16, kind="ExternalInput")
    out_d = nc.dram_tensor("y", [B, E], dt.float32, kind="ExternalOutput")

    with tile.TileContext(nc) as tc, ExitStack() as ctx:
        singles = ctx.enter_context(tc.tile_pool(name="singles", bufs=1))
        # Everything rides the two HWDGE rings (sync for the wlp stream,
        # scalar for wce/xpp/outputs): zero SWDGE traffic, so the SWDGE SBUF
        # descriptor rings that contend with SDMA engines 7/15 stay cold.
        wce = singles.tile([128, NECH * CIJ], dt.bfloat16)
        nc.scalar.dma_start(out=wce[:], in_=wce_d[:])
        xpp = singles.tile([CIJ, PL * B], dt.bfloat16)
        nc.scalar.dma_start(out=xpp[:, 0 : PL * B // 2], in_=xpp_d[:, 0 : PL * B // 2])
        nc.scalar.dma_start(out=xpp[:, PL * B // 2 :], in_=xpp_d[:, PL * B // 2 :])

        # W2 staging ring: one [49, 768] bf16 tile per patch, recycled.
        w2_pool = ctx.enter_context(tc.tile_pool(name="w2ring", bufs=6))
        wlp_pool = ctx.enter_context(tc.tile_pool(name="wlp", bufs=4))
        out_pool = ctx.enter_context(tc.tile_pool(name="out_sb", bufs=1))

        # Warmup: absorb the wce DMA-ready wait on a throwaway matmul so the
        # first real W2 matmul carries only the wlp(0) wait.
        with tc.tile_pool(name="psum_w", bufs=1, space="PSUM") as pwarm:
            wm = pwarm.tile([CIJ, CIJ], dt.float32)
            nc.tensor.matmul(
                wm[:], wce[:, 0:CIJ], wce[:, 0:CIJ], start=True, stop=True,
                skip_group_check=True,
            )

        with (
            tc.tile_pool(name="psum_y", bufs=1, space="PSUM") as pyp,
            tc.tile_pool(name="psum_w2", bufs=1, space="PSUM") as ppw,
        ):
            # y-partial accumulators: [128b x 512eo] + [128b x 256eo] per
            # b-half; each tile is a full PSUM bank.
            py = [
                [
                    pyp.tile([128, 512], dt.float32, tag=f"py{bh}0", name=f"py{bh}0"),
                    pyp.tile([128, 512], dt.float32, tag=f"py{bh}1", name=f"py{bh}1"),
                ]
                for bh in range(2)
            ]

            w2tiles = {}
            # Four bank-sized W2 accumulators (two patches in flight); bufs=1
            # per tag: pt+1's start=True reuse waits only on pt's copy.
            pw = [
                [
                    ppw.tile([CIJ, 512], dt.float32, tag=f"pa{j}", name=f"pa{j}"),
                    ppw.tile([CIJ, 512], dt.float32, tag=f"pb{j}", name=f"pb{j}"),
                ]
                for j in range(2)
            ]

            def w2_pair_block(pt, wl):
                # One LDWEIGHTS per (pt, ech): the four matmuls (2 patches x
                # 2 eo-slices) share the stationary wce e-chunk, so walrus
                # ldw-opt elides the redundant loads.
                for ech in range(NECH):
                    lhsT = wce[:, ech * CIJ : (ech + 1) * CIJ]
                    for j in range(2):
                        base = j * PCOL
                        nc.tensor.matmul(
                            pw[j][0][:, 0:512],
                            lhsT,
                            wl[:, base + ech * E : base + ech * E + 512],
                            start=(ech == 0),
                            stop=(ech == NECH - 1),
                            skip_group_check=True,
                        )
                        nc.tensor.matmul(
                            pw[j][1][:, 0:256],
                            lhsT,
                            wl[:, base + ech * E + 512 : base + ech * E + 768],
                            start=(ech == 0),
                            stop=(ech == NECH - 1),
                            skip_group_check=True,
                        )
                # PSUM f32 -> SBUF bf16 cast-copies; one engine per patch.
                for j in range(2):
                    p = 2 * pt + j
                    w2t = w2_pool.tile([CIJ, E], dt.bfloat16)
                    w2tiles[p] = w2t
                    if j == 0:
                        nc.vector.tensor_copy(w2t[:, 0:512], pw[j][0][:, 0:512])
                        nc.vector.tensor_copy(w2t[:, 512:768], pw[j][1][:, 0:256])
                    else:
                        nc.scalar.copy(w2t[:, 0:512], pw[j][0][:, 0:512])
                        nc.scalar.copy(w2t[:, 512:768], pw[j][1][:, 0:256])

            def final_block(p):
                w2t = w2tiles.pop(p)
                for bh in range(2):
                    lhsT = xpp[:, p * B + bh * 128 : p * B + bh * 128 + 128]
                    nc.tensor.matmul(
                        py[bh][0][:, 0:512],
                        lhsT,
                        w2t[:, 0:512],
                        start=(p == 0),
                        stop=(p == PL - 1),
                        skip_group_check=True,
                    )
                    nc.tensor.matmul(
                        py[bh][1][:, 0:256],
                        lhsT,
                        w2t[:, 512:768],
                        start=(p == 0),
                        stop=(p == PL - 1),
                        skip_group_check=True,
                    )

            # wlp streams as one sequential 2-patch DMA chain on the sync
            # HWDGE queue (18432B per-partition lines, one packet each); the
            # final blocks run one patch-pair behind W2 so the PE never
            # stalls on the PSUM->SBUF copy round-trip.
            for pt in range(PL // 2):
                wl = wlp_pool.tile([128, 2 * PCOL], dt.bfloat16)
                nc.sync.dma_start(
                    out=wl[:], in_=wlp_d[:, pt * 2 * PCOL : (pt + 1) * 2 * PCOL]
                )
                w2_pair_block(pt, wl)
                if pt >= 1:
                    final_block(2 * pt - 2)
                    final_block(2 * pt - 1)
            final_block(PL - 2)
            final_block(PL - 1)

            for bh in range(2):
                ob = out_pool.tile([128, E], dt.float32, tag=f"ob{bh}")
                nc.vector.tensor_copy(ob[:, 0:512], py[bh][0][:, 0:512])
                nc.scalar.copy(ob[:, 512:768], py[bh][1][:, 0:256])
                nc.scalar.dma_start(
                    out=out_d[bh * 128 : (bh + 1) * 128, :], in_=ob[:]
                )
    _split_extra_waits(nc)
    return nc


def _split_extra_waits(nc):
    """Walrus encodes at most one semaphore wait on regular engine
    instructions (Matmult, DMACopy, ...). When Tile attaches more (e.g.
    slot-recycle release + data-ready on different procs), split the extras
    onto InstEventSemaphore instructions inserted immediately before the
    instruction on the same engine queue -- semantically identical to the
    multi-wait (the engine blocks at the same point for all of them)."""
    import bass_rust
    import concourse.mybir as mybir

    keep_multi = {"InstEventSemaphore", "InstUnconditionalBranch"}
    n_split = 0
    for fn in nc.m.functions:
        for bb in fn.blocks:
            out = []
            changed = False
            for ins in bb.instructions:
                si = ins.sync_info
                if (
                    si is not None
                    and len(si.on_wait) > 1
                    and type(ins).__name__ not in keep_multi
                ):
                    waits = list(si.on_wait)
                    for w in waits[:-1]:
                        ev = mybir.InstEventSemaphore(
                            name=f"W-split-{n_split}", ins=[], outs=[]
                        )
                        n_split += 1
                        ev.engine = ins.engine
                        ev.sync_info = bass_rust.SyncInfo(on_wait=[w], on_update=[])
                        out.append(ev)
                    ins.sync_info = bass_rust.SyncInfo(
                        on_wait=[waits[-1]], on_update=list(si.on_update)
                    )
                    changed = True
                out.append(ins)
            if changed:
                bb.instructions = out
    return n_split


def _prep_inputs(x, wconv, bconv, wlin):
    bf16 = ml_dtypes.bfloat16
    x = np.ascontiguousarray(np.asarray(x, dtype=np.float32))
    wconv = np.asarray(wconv, dtype=np.float32)
    bconv = np.asarray(bconv, dtype=np.float32)
    wlin = np.asarray(wlin, dtype=np.float32)

    # im2col: xpa[(c,i,j), b, p] = x[b, c, 4hp+i, 4wp+j], p = hp*16+wp;
    # row 48 = ones (bias row). Pure index remap, zero FLOPs.
    xp = x.reshape(B, C, Hp, P, Wp, P).transpose(1, 3, 5, 0, 2, 4)
    xpa = np.empty((CIJ, B, NP), np.float32)
    xpa[:48] = xp.reshape(48, B, NP)
    xpa[48] = 1.0

    # wce[e_r, ech, cij] = wcaT[ech*128+e_r, cij]; wca row 48 = bconv.
    wca = np.empty((CIJ, E), np.float32)
    wca[:48] = wconv.reshape(E, 48).T
    wca[48] = bconv
    wce = np.ascontiguousarray(
        wca.T.reshape(NECH, 128, CIJ).transpose(1, 0, 2).reshape(128, NECH * CIJ)
    ).astype(bf16)

    wlinR = wlin.reshape(E, E, NP)  # [eo, e, p]
    in_maps = []
    for c in range(NCORES):
        ps = c * PL
        # wlp[e_r, p*4608 + ech*768 + eo] = wlin[eo, (ech*128+e_r)*256 + p]
        wlp = (
            wlinR[:, :, ps : ps + PL]
            .transpose(1, 2, 0)                 # [e, p, eo]
            .reshape(NECH, 128, PL, E)
            .transpose(1, 2, 0, 3)              # [e_r, p, ech, eo]
            .reshape(128, PL * PCOL)
            .astype(bf16)
        )
        xpp = (
            xpa[:, :, ps : ps + PL]
            .transpose(0, 2, 1)                 # [cij, p, b]
            .reshape(CIJ, PL * B)
            .astype(bf16)
        )
        in_maps.append({"wce": wce, "xpp": xpp, "wlp": wlp})
    return in_maps


def _patch_ldw_opt():
    """walrus is invoked with --enable-ldw-opt=false (hardcoded); enabling it
    lets codegen elide redundant LDWEIGHTS. Rewrite the flag on the way in."""
    from concourse import bass_utils as _bu

    if getattr(_bu, "_ldw_opt_patched", False):
        return
    _orig = _bu.run_command

    def _patched(cmd, **kw):
        if isinstance(cmd, list):
            cmd = [
                "--enable-ldw-opt=true" if c == "--enable-ldw-opt=false" else c
                for c in cmd
            ]
        return _orig(cmd, **kw)

    _bu.run_command = _patched
    _bu._ldw_opt_patched = True


def _run(x, wconv, bconv, wlin, blin, trace=False, **trace_kwargs):
    from concourse.bass_utils import run_bass_kernel_spmd

    if "nc" not in _CACHE:
        _CACHE["nc"] = _build_bass()
    in_maps = _prep_inputs(x, wconv, bconv, wlin)
    res = run_bass_kernel_spmd(
        _CACHE["nc"], in_maps, core_ids=list(range(NCORES)), trace=trace,
        **trace_kwargs,
    )
    acc = np.zeros((B, E), np.float64)
    for r in res.results:
        acc += r["y"]
    y = (acc + np.asarray(blin, dtype=np.float64)[None, :]).astype(np.float32)
    return y, res


def kernel(x, wconv, bconv, wlin, blin, patch_size):
    assert int(patch_size) == P
    y, _ = _run(x, wconv, bconv, wlin, blin, trace=False)
    return y


# revision 20
# speedup vs baseline: 1.3230x; 1.0664x over previous
"""Trainium2 Bass kernel: patch-conv (Conv2d C3->E768, k4 s4) + giant linear.

y[b, eo] = sum_K flat[b, K] * wlin[eo, K] + blin[eo],
flat[b, e*256+p] = conv[b, e, p] (+ bconv[e]), K = 196608.

Reassociated algorithm (matrix-chain reordering, all FLOPs on device):
    y[b,eo] = sum_{p,cij} xp[cij,p,b] * W2[p,cij,eo]
    W2[p,cij,eo] = sum_e wca[cij,e] * wlin[eo, e*256+p]
where xp is the im2col of x (pure index remap, row 48 = ones) and wca is
wconv reshaped [49, E] (row 48 = bconv). This computes the identical
function with 19.7 GFLOP instead of 82 GFLOP, and reads wlin exactly once.

Sharding (8 cores): shard the patch index p (32 patches/core). Each core:
  - reads its wlin slice re-laid-out on host as wlp[e_r, (p, ech, eo)] bf16
    (37.75 MB -- the DMA roofline term),
  - W2-mm: for each p: 6 e-chunks x (N=512 + N=256) matmuls, stationary
    wce[e_chunk] = wcaT slice [128,49], accumulate PSUM [49,768],
  - copies W2_p to SBUF bf16 (vector/scalar alternating),
  - final-mm: stationary xpp[:, p, b-half] [49,128], moving W2_p, PSUM
    accumulates y-partial [128b, 512|256 eo] over the 32 p's.
  - Host: sum the 8 partial y's, add blin.
All PSUM tiles are bank-sized (2048B) so accumulation groups never cross
a PSUM bank: 4 banks y-accum + 2x2 banks W2-accum = 8.
"""

import numpy as np
import ml_dtypes

B, C, H, W = 256, 3, 64, 64
P, Hp, Wp, NP = 4, 16, 16, 256
E = 768
CIJ = 49                  # 48 conv taps + 1 bias row
NCORES = 8
PL = NP // NCORES         # 32 patches per core
NECH = E // 128           # 6 e-chunks in the W2 contraction
PCOL = NECH * E           # 4608 wlp columns per patch

_CACHE = {}


def _build_bass():
    import concourse.bass as bass
    import concourse.mybir as mybir
    import concourse.tile as tile
    from contextlib import ExitStack

    dt = mybir.dt
    nc = bass.Bass()
    wce_d = nc.dram_tensor("wce", [128, NECH * CIJ], dt.bfloat16, kind="ExternalInput")
    xpp_d = nc.dram_tensor("xpp", [CIJ, PL * B], dt.bfloat16, kind="ExternalInput")
    wlp_d = nc.dram_tensor("wlp", [128, PL * PCOL], dt.bfloat16, kind="ExternalInput")
    out_d = nc.dram_tensor("y", [B, E], dt.float32, kind="ExternalOutput")

    with tile.TileContext(nc) as tc, ExitStack() as ctx:
        singles = ctx.enter_context(tc.tile_pool(name="singles", bufs=1))
        # Everything rides the two HWDGE rings (sync for the wlp stream,
        # scalar for wce/xpp/outputs): zero SWDGE traffic, so the SWDGE SBUF
        # descriptor rings that contend with SDMA engines 7/15 stay cold.
        wce = singles.tile([128, NECH * CIJ], dt.bfloat16)
        nc.scalar.dma_start(out=wce[:], in_=wce_d[:])
        xpp = singles.tile([CIJ, PL * B], dt.bfloat16)
        nc.scalar.dma_start(out=xpp[:, 0 : PL * B // 2], in_=xpp_d[:, 0 : PL * B // 2])
        nc.scalar.dma_start(out=xpp[:, PL * B // 2 :], in_=xpp_d[:, PL * B // 2 :])

        # W2 staging ring: one [49, 768] bf16 tile per patch, recycled.
        w2_pool = ctx.enter_context(tc.tile_pool(name="w2ring", bufs=6))
        wlp_pool = ctx.enter_context(tc.tile_pool(name="wlp", bufs=4))
        out_pool = ctx.enter_context(tc.tile_pool(name="out_sb", bufs=1))

        # Warmup: absorb the wce DMA-ready wait on a throwaway matmul so the
        # first real W2 matmul carries only the wlp(0) wait.
        with tc.tile_pool(name="psum_w", bufs=1, space="PSUM") as pwarm:
            wm = pwarm.tile([CIJ, CIJ], dt.float32)
            nc.tensor.matmul(
                wm[:], wce[:, 0:CIJ], wce[:, 0:CIJ], start=True, stop=True,
                skip_group_check=True,
            )

        with (
            tc.tile_pool(name="psum_y", bufs=1, space="PSUM") as pyp,
            tc.tile_pool(name="psum_w2", bufs=1, space="PSUM") as ppw,
        ):
            # y-partial accumulators: [128b x 512eo] + [128b x 256eo] per
            # b-half; each tile is a full PSUM bank.
            py = [
                [
                    pyp.tile([128, 512], dt.float32, tag=f"py{bh}0", name=f"py{bh}0"),
                    pyp.tile([128, 512], dt.float32, tag=f"py{bh}1", name=f"py{bh}1"),
                ]
                for bh in range(2)
            ]

            w2tiles = {}
            # Four bank-sized W2 accumulators (two patches in flight); bufs=1
            # per tag: pt+1's start=True reuse waits only on pt's copy.
            pw = [
                [
                    ppw.tile([CIJ, 512], dt.float32, tag=f"pa{j}", name=f"pa{j}"),
                    ppw.tile([CIJ, 512], dt.float32, tag=f"pb{j}", name=f"pb{j}"),
                ]
                for j in range(2)
            ]

            def w2_pair_block(pt, wl):
                # One LDWEIGHTS per (pt, ech): the four matmuls (2 patches x
                # 2 eo-slices) share the stationary wce e-chunk, so walrus
                # ldw-opt elides the redundant loads.
                for ech in range(NECH):
                    lhsT = wce[:, ech * CIJ : (ech + 1) * CIJ]
                    for j in range(2):
                        base = j * PCOL
                        nc.tensor.matmul(
                            pw[j][0][:, 0:512],
                            lhsT,
                            wl[:, base + ech * E : base + ech * E + 512],
                            start=(ech == 0),
                            stop=(ech == NECH - 1),
                            skip_group_check=True,
                        )
                        nc.tensor.matmul(
                            pw[j][1][:, 0:256],
                            lhsT,
                            wl[:, base + ech * E + 512 : base + ech * E + 768],
                            start=(ech == 0),
                            stop=(ech == NECH - 1),
                            skip_group_check=True,
                        )
                # PSUM f32 -> SBUF bf16 cast-copies; one engine per patch.
                for j in range(2):
                    p = 2 * pt + j
                    w2t = w2_pool.tile([CIJ, E], dt.bfloat16)
                    w2tiles[p] = w2t
                    if j == 0:
                        nc.vector.tensor_copy(w2t[:, 0:512], pw[j][0][:, 0:512])
                        nc.vector.tensor_copy(w2t[:, 512:768], pw[j][1][:, 0:256])
                    else:
                        nc.scalar.copy(w2t[:, 0:512], pw[j][0][:, 0:512])
                        nc.scalar.copy(w2t[:, 512:768], pw[j][1][:, 0:256])

            def final_block(p):
                w2t = w2tiles.pop(p)
                for bh in range(2):
                    lhsT = xpp[:, p * B + bh * 128 : p * B + bh * 128 + 128]
                    nc.tensor.matmul(
                        py[bh][0][:, 0:512],
                        lhsT,
                        w2t[:, 0:512],
                        start=(p == 0),
                        stop=(p == PL - 1),
                        skip_group_check=True,
                    )
                    nc.tensor.matmul(
                        py[bh][1][:, 0:256],
                        lhsT,
                        w2t[:, 512:768],
                        start=(p == 0),
                        stop=(p == PL - 1),
                        skip_group_check=True,
                    )

            def w2_single_block(p, j):
                # 1-patch variant for the stream tail: shortest possible
                # serial chain after the last wlp packet lands.
                wl = wlp_pool.tile([128, 2 * PCOL], dt.bfloat16, name="wl")
                nc.sync.dma_start(
                    out=wl[:, 0:PCOL], in_=wlp_d[:, p * PCOL : (p + 1) * PCOL]
                )
                for ech in range(NECH):
                    lhsT = wce[:, ech * CIJ : (ech + 1) * CIJ]
                    nc.tensor.matmul(
                        pw[j][0][:, 0:512],
                        lhsT,
                        wl[:, ech * E : ech * E + 512],
                        start=(ech == 0),
                        stop=(ech == NECH - 1),
                        skip_group_check=True,
                    )
                    nc.tensor.matmul(
                        pw[j][1][:, 0:256],
                        lhsT,
                        wl[:, ech * E + 512 : ech * E + 768],
                        start=(ech == 0),
                        stop=(ech == NECH - 1),
                        skip_group_check=True,
                    )
                w2t = w2_pool.tile([CIJ, E], dt.bfloat16)
                w2tiles[p] = w2t
                if j == 0:
                    nc.vector.tensor_copy(w2t[:, 0:512], pw[j][0][:, 0:512])
                    nc.vector.tensor_copy(w2t[:, 512:768], pw[j][1][:, 0:256])
                else:
                    nc.scalar.copy(w2t[:, 0:512], pw[j][0][:, 0:512])
                    nc.scalar.copy(w2t[:, 512:768], pw[j][1][:, 0:256])

            # wlp streams as one sequential 2-patch DMA chain on the sync
            # HWDGE queue (18432B per-partition lines, one packet each); the
            # final blocks run one patch-pair behind W2 so the PE never
            # stalls on the PSUM->SBUF copy round-trip. The last pair is
            # split into 1-patch DMAs to shorten the end-of-stream chain.
            for pt in range(PL // 2 - 1):
                wl = wlp_pool.tile([128, 2 * PCOL], dt.bfloat16, name="wl")
                nc.sync.dma_start(
                    out=wl[:], in_=wlp_d[:, pt * 2 * PCOL : (pt + 1) * 2 * PCOL]
                )
                w2_pair_block(pt, wl)
                if pt >= 1:
                    final_block(2 * pt - 2)
                    final_block(2 * pt - 1)
            w2_single_block(PL - 2, 0)
            final_block(PL - 4)
            final_block(PL - 3)
            w2_single_block(PL - 1, 1)
            final_block(PL - 2)
            final_block(PL - 1)

            for bh in range(2):
                ob = out_pool.tile([128, E], dt.float32, tag=f"ob{bh}")
                nc.vector.tensor_copy(ob[:, 0:512], py[bh][0][:, 0:512])
                nc.scalar.copy(ob[:, 512:768], py[bh][1][:, 0:256])
                nc.scalar.dma_start(
                    out=out_d[bh * 128 : (bh + 1) * 128, :], in_=ob[:]
                )
    _split_extra_waits(nc)
    return nc


def _split_extra_waits(nc):
    """Walrus encodes at most one semaphore wait on regular engine
    instructions (Matmult, DMACopy, ...). When Tile attaches more (e.g.
    slot-recycle release + data-ready on different procs), split the extras
    onto InstEventSemaphore instructions inserted immediately before the
    instruction on the same engine queue -- semantically identical to the
    multi-wait (the engine blocks at the same point for all of them)."""
    import bass_rust
    import concourse.mybir as mybir

    keep_multi = {"InstEventSemaphore", "InstUnconditionalBranch"}
    n_split = 0
    for fn in nc.m.functions:
        for bb in fn.blocks:
            out = []
            changed = False
            for ins in bb.instructions:
                si = ins.sync_info
                if (
                    si is not None
                    and len(si.on_wait) > 1
                    and type(ins).__name__ not in keep_multi
                ):
                    waits = list(si.on_wait)
                    for w in waits[:-1]:
                        ev = mybir.InstEventSemaphore(
                            name=f"W-split-{n_split}", ins=[], outs=[]
                        )
                        n_split += 1
                        ev.engine = ins.engine
                        ev.sync_info = bass_rust.SyncInfo(on_wait=[w], on_update=[])
                        out.append(ev)
                    ins.sync_info = bass_rust.SyncInfo(
                        on_wait=[waits[-1]], on_update=list(si.on_update)
                    )
                    changed = True
                out.append(ins)
            if changed:
                bb.instructions = out
    return n_split


def _prep_inputs(x, wconv, bconv, wlin):
    bf16 = ml_dtypes.bfloat16
    x = np.ascontiguousarray(np.asarray(x, dtype=np.float32))
    wconv = np.asarray(wconv, dtype=np.float32)
    bconv = np.asarray(bconv, dtype=np.float32)
    wlin = np.asarray(wlin, dtype=np.float32)

    # im2col: xpa[(c,i,j), b, p] = x[b, c, 4hp+i, 4wp+j], p = hp*16+wp;
    # row 48 = ones (bias row). Pure index remap, zero FLOPs.
    xp = x.reshape(B, C, Hp, P, Wp, P).transpose(1, 3, 5, 0, 2, 4)
    xpa = np.empty((CIJ, B, NP), np.float32)
    xpa[:48] = xp.reshape(48, B, NP)
    xpa[48] = 1.0

    # wce[e_r, ech, cij] = wcaT[ech*128+e_r, cij]; wca row 48 = bconv.
    wca = np.empty((CIJ, E), np.float32)
    wca[:48] = wconv.reshape(E, 48).T
    wca[48] = bconv
    wce = np.ascontiguousarray(
        wca.T.reshape(NECH, 128, CIJ).transpose(1, 0, 2).reshape(128, NECH * CIJ)
    ).astype(bf16)

    wlinR = wlin.reshape(E, E, NP)  # [eo, e, p]
    in_maps = []
    for c in range(NCORES):
        ps = c * PL
        # wlp[e_r, p*4608 + ech*768 + eo] = wlin[eo, (ech*128+e_r)*256 + p]
        wlp = (
            wlinR[:, :, ps : ps + PL]
            .transpose(1, 2, 0)                 # [e, p, eo]
            .reshape(NECH, 128, PL, E)
            .transpose(1, 2, 0, 3)              # [e_r, p, ech, eo]
            .reshape(128, PL * PCOL)
            .astype(bf16)
        )
        xpp = (
            xpa[:, :, ps : ps + PL]
            .transpose(0, 2, 1)                 # [cij, p, b]
            .reshape(CIJ, PL * B)
            .astype(bf16)
        )
        in_maps.append({"wce": wce, "xpp": xpp, "wlp": wlp})
    return in_maps


def _patch_ldw_opt():
    """walrus is invoked with --enable-ldw-opt=false (hardcoded); enabling it
    lets codegen elide redundant LDWEIGHTS. Rewrite the flag on the way in."""
    from concourse import bass_utils as _bu

    if getattr(_bu, "_ldw_opt_patched", False):
        return
    _orig = _bu.run_command

    def _patched(cmd, **kw):
        if isinstance(cmd, list):
            cmd = [
                "--enable-ldw-opt=true" if c == "--enable-ldw-opt=false" else c
                for c in cmd
            ]
        return _orig(cmd, **kw)

    _bu.run_command = _patched
    _bu._ldw_opt_patched = True


def _run(x, wconv, bconv, wlin, blin, trace=False, **trace_kwargs):
    from concourse.bass_utils import run_bass_kernel_spmd

    if "nc" not in _CACHE:
        _CACHE["nc"] = _build_bass()
    in_maps = _prep_inputs(x, wconv, bconv, wlin)
    res = run_bass_kernel_spmd(
        _CACHE["nc"], in_maps, core_ids=list(range(NCORES)), trace=trace,
        **trace_kwargs,
    )
    acc = np.zeros((B, E), np.float64)
    for r in res.results:
        acc += r["y"]
    y = (acc + np.asarray(blin, dtype=np.float64)[None, :]).astype(np.float32)
    return y, res


def kernel(x, wconv, bconv, wlin, blin, patch_size):
    assert int(patch_size) == P
    y, _ = _run(x, wconv, bconv, wlin, blin, trace=False)
    return y
